# revision 17
# baseline (speedup 1.0000x reference)
import sys

sys.path.insert(0, "/opt/trn_rl_repo")

import numpy as np
import ml_dtypes

from concourse import bass, bacc, tile, mybir
from concourse.bass_utils import run_bass_kernel_spmd

B, S, N, D = 4, 96, 512, 8
H = 64
OUT = 24
NT = N // 128  # 4 node tiles of 128 partitions
F = H + D     # 72 features in v = [h | x]
FB = F + 1    # +1 ones row for bias

BF16 = mybir.dt.bfloat16
FP32 = mybir.dt.float32

_CACHE = {}


def _build_nc():
    nc = bacc.Bacc(None)
    adjT_d = nc.dram_tensor("adjT", [S, 128, NT, N], BF16, kind="ExternalInput")
    xT_d = nc.dram_tensor("xT", [128, S, NT, D], BF16, kind="ExternalInput")
    wb_d = nc.dram_tensor("wb", [FB, 4 * H], BF16, kind="ExternalInput")
    h0_d = nc.dram_tensor("h0T", [128, NT, H], BF16, kind="ExternalInput")
    c0_d = nc.dram_tensor("c0T", [128, NT, H], FP32, kind="ExternalInput")
    hout_d = nc.dram_tensor("hout", [128, NT, H], FP32, kind="ExternalOutput")

    with tile.TileContext(nc) as tc:
        with (
            tc.tile_pool(name="persist", bufs=1) as persist,
            tc.tile_pool(name="adj", bufs=3) as adjp,
            tc.tile_pool(name="scratch", bufs=2) as scratch,
            tc.tile_pool(name="ps_av", bufs=2, space="PSUM") as ps_av,
            tc.tile_pool(name="ps_g", bufs=2, space="PSUM") as ps_g,
        ):
            X = persist.tile([128, S, NT, D], BF16)   # all timesteps of x
            V = persist.tile([128, NT, F], BF16)      # [h | x] per node tile
            C = persist.tile([128, NT, H], FP32)      # cell state
            WB = persist.tile([FB, 4 * H], BF16)      # [Wh; Wx; b]
            AVT = persist.tile([FB, N], BF16)         # Av^T + ones row
            HF = persist.tile([128, NT, H], FP32)     # final h, fp32

            H0 = persist.tile([128, NT, H], BF16)

            nc.gpsimd.dma_start(X[:], xT_d[:])
            nc.gpsimd.dma_start(WB[:], wb_d[:])
            nc.gpsimd.dma_start(H0[:], h0_d[:])
            nc.gpsimd.dma_start(C[:], c0_d[:])
            # all V producers stay on DVE so matmul LDW needs a single wait
            nc.vector.tensor_copy(V[:, :, 0:H], H0[:])
            # ones row (72) for bias; partition offset must be mult of 32, so
            # memset 64:73 once — rows 64:72 are rewritten with data each step.
            nc.vector.memset(AVT[64:FB, :], 1.0)

            for s in range(S):
                AT = adjp.tile([128, NT, N], BF16, name="AT", tag="AT")
                nc.sync.dma_start(AT[:], adjT_d[s])

                # x_s into V x slots (SBUF -> SBUF)
                nc.vector.tensor_copy(V[:, :, H : H + D], X[:, s, :, :])

                # mm1 in two n-halves: half-0 cast (ACT) overlaps half-1 MMs.
                # Separate PSUM tiles per half — one shared tile makes the
                # tracker serialize half-1's writes behind half-0's cast read.
                AvT0 = ps_av.tile([FB, 256], FP32, name="AvT0", tag="AvT0")
                AvT1 = ps_av.tile([FB, 256], FP32, name="AvT1", tag="AvT1")
                G = ps_g.tile([128, NT, 4 * H], FP32, name="G", tag="G")
                for mt in range(NT):
                    nc.tensor.matmul(
                        AvT0[0:F, :],
                        V[:, mt, :],
                        AT[:, mt, 0:256],
                        start=(mt == 0),
                        stop=(mt == NT - 1),
                    )
                nc.scalar.activation(
                    AVT[0:F, 0:256], AvT0[0:F, :],
                    mybir.ActivationFunctionType.Copy,
                )
                for mt in range(NT):
                    nc.tensor.matmul(
                        AvT1[0:F, :],
                        V[:, mt, :],
                        AT[:, mt, 256:512],
                        start=(mt == 0),
                        stop=(mt == NT - 1),
                    )
                # half-1 cast split ACT/DVE so mm2 nt2/nt3 aren't cast-limited
                nc.scalar.activation(
                    AVT[0:F, 256:384], AvT1[0:F, 0:128],
                    mybir.ActivationFunctionType.Copy,
                )
                nc.vector.tensor_copy(AVT[0:F, 384:512], AvT1[0:F, 128:256])
                for nt in range(NT):
                    nc.tensor.matmul(
                        G[:, nt, :],
                        AVT[:, nt * 128 : (nt + 1) * 128],
                        WB[:],
                        start=True,
                        stop=True,
                    )

                SIF = scratch.tile([128, NT, 2 * H], BF16, name="SIF", tag="SIF")
                TG = scratch.tile([128, NT, H], BF16, name="TG", tag="TG")
                SO = scratch.tile([128, NT, H], BF16, name="SO", tag="SO")
                # i,f gates are contiguous cols 0:2H -> one fused sigmoid, so
                # tanh(gg) (critical for IG) finishes one ACT slot earlier.
                nc.scalar.activation(
                    SIF[:], G[:, :, 0 : 2 * H], mybir.ActivationFunctionType.Sigmoid
                )
                nc.scalar.activation(
                    TG[:], G[:, :, 3 * H : 4 * H], mybir.ActivationFunctionType.Tanh
                )
                nc.scalar.activation(
                    SO[:], G[:, :, 2 * H : 3 * H], mybir.ActivationFunctionType.Sigmoid
                )

                IG = scratch.tile([128, NT, H], BF16, name="IG", tag="IG")
                FC = scratch.tile([128, NT, H], FP32, name="FC", tag="FC")
                TC = scratch.tile([128, NT, H], BF16, name="TC", tag="TC")

                # FC = f*c on gpsimd (stride-free); IG = i*gg split DVE/GP so
                # C_a starts right after the DVE half instead of the full IG
                nc.gpsimd.tensor_tensor(
                    FC[:], SIF[:, :, H : 2 * H], C[:], mybir.AluOpType.mult
                )
                nc.vector.scalar_tensor_tensor(
                    IG[:, 0:2, :], SIF[:, 0:2, 0:H], 1.0, TG[:, 0:2, :],
                    mybir.AluOpType.bypass, mybir.AluOpType.mult,
                )
                nc.gpsimd.tensor_tensor(
                    IG[:, 2:NT, :], SIF[:, 2:NT, 0:H], TG[:, 2:NT, :],
                    mybir.AluOpType.mult,
                )
                # split C/TC/h into nt-halves so DVE and ACT pipeline the tail
                nc.vector.scalar_tensor_tensor(
                    C[:, 0:2, :], FC[:, 0:2, :], 1.0, IG[:, 0:2, :],
                    mybir.AluOpType.bypass, mybir.AluOpType.add,
                )
                nc.vector.scalar_tensor_tensor(
                    C[:, 2:NT, :], FC[:, 2:NT, :], 1.0, IG[:, 2:NT, :],
                    mybir.AluOpType.bypass, mybir.AluOpType.add,
                )
                nc.scalar.activation(
                    TC[:, 0:2, :], C[:, 0:2, :], mybir.ActivationFunctionType.Tanh
                )
                nc.scalar.activation(
                    TC[:, 2:NT, :], C[:, 2:NT, :], mybir.ActivationFunctionType.Tanh
                )
                if s == S - 1:
                    nc.vector.scalar_tensor_tensor(
                        HF[:], SO[:], 1.0, TC[:],
                        mybir.AluOpType.bypass, mybir.AluOpType.mult,
                    )
                else:
                    nc.vector.scalar_tensor_tensor(
                        V[:, 0:2, 0:H], SO[:, 0:2, :], 1.0, TC[:, 0:2, :],
                        mybir.AluOpType.bypass, mybir.AluOpType.mult,
                    )
                    nc.vector.scalar_tensor_tensor(
                        V[:, 2:NT, 0:H], SO[:, 2:NT, :], 1.0, TC[:, 2:NT, :],
                        mybir.AluOpType.bypass, mybir.AluOpType.mult,
                    )

            nc.sync.dma_start(hout_d[:], HF[:])

    nc.finalize()  # Bacc.finalize runs the multi-wait-splitting passes
    return nc


def _prep_core_inputs(b, x, adj, h0, c0, Wh, Wx, b_gates):
    bf16 = ml_dtypes.bfloat16
    # adjT[s, p, mt, n] = adj[b, s, n, mt*128+p]  (= A_s^T row m, col n)
    a = adj[b].transpose(0, 2, 1).reshape(S, NT, 128, N).transpose(0, 2, 1, 3)
    adjT = np.ascontiguousarray(a, dtype=bf16)
    # xT[p, s, mt, d] = x[b, s, mt*128+p, d]
    xb = x[b].reshape(S, NT, 128, D).transpose(2, 0, 1, 3)
    xT = np.ascontiguousarray(xb, dtype=bf16)
    # h0T/c0T[p, nt, j] = state[b, nt*128+p, j]
    h0b = h0[b].reshape(NT, 128, H).transpose(1, 0, 2)
    c0b = c0[b].reshape(NT, 128, H).transpose(1, 0, 2)
    h0T = np.ascontiguousarray(h0b, dtype=bf16)
    c0T = np.ascontiguousarray(c0b, dtype=np.float32)
    wb = np.concatenate([Wh, Wx, b_gates[None, :]], axis=0).astype(np.float32)
    wb16 = wb.astype(bf16)
    return {"adjT": adjT, "xT": xT, "wb": wb16, "h0T": h0T, "c0T": c0T}


def kernel(x, adj, initial_hidden_state, initial_cell_state, Wx, Wh, b_gates,
           W1, b1, W2, b2):
    x = np.asarray(x, dtype=np.float32)
    adj = np.asarray(adj, dtype=np.float32)
    h0 = np.asarray(initial_hidden_state, dtype=np.float32)
    c0 = np.asarray(initial_cell_state, dtype=np.float32)
    Wx_ = np.asarray(Wx, dtype=np.float32)
    Wh_ = np.asarray(Wh, dtype=np.float32)
    bg = np.asarray(b_gates, dtype=np.float32)

    if "nc" not in _CACHE:
        _CACHE["nc"] = _build_nc()
    nc = _CACHE["nc"]

    core_ids = list(range(B))
    in_maps = [_prep_core_inputs(b, x, adj, h0, c0, Wh_, Wx_, bg) for b in range(B)]
    res = run_bass_kernel_spmd(nc, in_maps, core_ids)

    h_final = np.zeros((B, N, H), dtype=np.float32)
    for i in range(B):
        hout = np.asarray(res.results[i]["hout"], dtype=np.float32)  # [128, NT, H]
        h_final[i] = hout.transpose(1, 0, 2).reshape(N, H)

    read_out = h_final[:, 0, :]  # (B, H) -- TARGET_NODE = 0
    pre = read_out @ np.asarray(W1, dtype=np.float32) + np.asarray(b1, dtype=np.float32)
    out = np.maximum(pre, 0.0) @ np.asarray(W2, dtype=np.float32) + np.asarray(
        b2, dtype=np.float32
    )
    return out.astype(np.float32)


# revision 20
# speedup vs baseline: 1.0389x; 1.0389x over previous
import sys

sys.path.insert(0, "/opt/trn_rl_repo")

import numpy as np
import ml_dtypes

from concourse import bass, bacc, tile, mybir
from concourse.bass_utils import run_bass_kernel_spmd

B, S, N, D = 4, 96, 512, 8
H = 64
OUT = 24
NT = N // 128  # 4 node tiles of 128 partitions
F = H + D     # 72 features in v = [h | x]
FB = F + 1    # +1 ones row for bias

BF16 = mybir.dt.bfloat16
FP32 = mybir.dt.float32

_CACHE = {}


def _build_nc():
    nc = bacc.Bacc(None)
    adjT_d = nc.dram_tensor("adjT", [S, 128, NT, N], BF16, kind="ExternalInput")
    xT_d = nc.dram_tensor("xT", [128, S, NT, D], BF16, kind="ExternalInput")
    wb_d = nc.dram_tensor("wb", [FB, 4 * H], BF16, kind="ExternalInput")
    h0_d = nc.dram_tensor("h0T", [128, NT, H], BF16, kind="ExternalInput")
    c0_d = nc.dram_tensor("c0T", [128, NT, H], FP32, kind="ExternalInput")
    hout_d = nc.dram_tensor("hout", [128, NT, H], FP32, kind="ExternalOutput")

    with tile.TileContext(nc) as tc:
        with (
            tc.tile_pool(name="persist", bufs=1) as persist,
            tc.tile_pool(name="adj", bufs=3) as adjp,
            tc.tile_pool(name="scratch", bufs=2) as scratch,
            tc.tile_pool(name="ps_av", bufs=2, space="PSUM") as ps_av,
            tc.tile_pool(name="ps_g", bufs=2, space="PSUM") as ps_g,
        ):
            X = persist.tile([128, S, NT, D], BF16)   # all timesteps of x
            V = persist.tile([128, NT, F], BF16)      # [h | x] per node tile
            C = persist.tile([128, NT, H], FP32)      # cell state
            WB = persist.tile([FB, 4 * H], BF16)      # [Wh; Wx; b]
            AVT = persist.tile([FB, N], BF16)         # Av^T + ones row
            HF = persist.tile([128, NT, H], FP32)     # final h, fp32

            H0 = persist.tile([128, NT, H], BF16)

            nc.gpsimd.dma_start(X[:], xT_d[:])
            nc.gpsimd.dma_start(WB[:], wb_d[:])
            nc.gpsimd.dma_start(H0[:], h0_d[:])
            nc.gpsimd.dma_start(C[:], c0_d[:])
            # all V producers stay on DVE so matmul LDW needs a single wait
            nc.vector.tensor_copy(V[:, :, 0:H], H0[:])
            # ones row (72) for bias; partition offset must be mult of 32, so
            # memset 64:73 once — rows 64:72 are rewritten with data each step.
            nc.vector.memset(AVT[64:FB, :], 1.0)

            for s in range(S):
                AT = adjp.tile([128, NT, N], BF16, name="AT", tag="AT")
                nc.sync.dma_start(AT[:], adjT_d[s])

                # x_s into V x slots (SBUF -> SBUF)
                nc.vector.tensor_copy(V[:, :, H : H + D], X[:, s, :, :])

                # mm1 in two n-halves: half-0 cast (ACT) overlaps half-1 MMs.
                # Separate PSUM tiles per half — one shared tile makes the
                # tracker serialize half-1's writes behind half-0's cast read.
                AvT0 = ps_av.tile([FB, 256], FP32, name="AvT0", tag="AvT0")
                AvT1 = ps_av.tile([FB, 256], FP32, name="AvT1", tag="AvT1")
                G = ps_g.tile([128, NT, 4 * H], FP32, name="G", tag="G")
                for mt in range(NT):
                    nc.tensor.matmul(
                        AvT0[0:F, :],
                        V[:, mt, :],
                        AT[:, mt, 0:256],
                        start=(mt == 0),
                        stop=(mt == NT - 1),
                    )
                nc.scalar.activation(
                    AVT[0:F, 0:256], AvT0[0:F, :],
                    mybir.ActivationFunctionType.Copy,
                )
                for mt in range(NT):
                    nc.tensor.matmul(
                        AvT1[0:F, :],
                        V[:, mt, :],
                        AT[:, mt, 256:512],
                        start=(mt == 0),
                        stop=(mt == NT - 1),
                    )
                # half-1 cast split ACT/DVE so mm2 nt2/nt3 aren't cast-limited
                nc.scalar.activation(
                    AVT[0:F, 256:384], AvT1[0:F, 0:128],
                    mybir.ActivationFunctionType.Copy,
                )
                nc.vector.tensor_copy(AVT[0:F, 384:512], AvT1[0:F, 128:256])
                for nt in range(2):
                    nc.tensor.matmul(
                        G[:, nt, :],
                        AVT[:, nt * 128 : (nt + 1) * 128],
                        WB[:],
                        start=True,
                        stop=True,
                    )

                SIFO = scratch.tile([128, NT, 3 * H], BF16, name="SIFO", tag="SIFO")
                TG = scratch.tile([128, NT, H], BF16, name="TG", tag="TG")
                IG = scratch.tile([128, NT, H], BF16, name="IG", tag="IG")
                FC = scratch.tile([128, NT, H], FP32, name="FC", tag="FC")
                TC = scratch.tile([128, NT, H], BF16, name="TC", tag="TC")

                # half A (nts 0:2) gates start right after mm2 nt1, while mm2
                # nt2/nt3 still run on PE; i,f,o contiguous -> fused sigmoid
                nc.scalar.activation(
                    SIFO[:, 0:2, :], G[:, 0:2, 0 : 3 * H],
                    mybir.ActivationFunctionType.Sigmoid,
                )
                nc.scalar.activation(
                    TG[:, 0:2, :], G[:, 0:2, 3 * H : 4 * H],
                    mybir.ActivationFunctionType.Tanh,
                )
                nc.gpsimd.tensor_tensor(
                    FC[:, 0:2, :], SIFO[:, 0:2, H : 2 * H], C[:, 0:2, :],
                    mybir.AluOpType.mult,
                )
                nc.vector.scalar_tensor_tensor(
                    IG[:, 0:2, :], SIFO[:, 0:2, 0:H], 1.0, TG[:, 0:2, :],
                    mybir.AluOpType.bypass, mybir.AluOpType.mult,
                )
                for nt in range(2, NT):
                    nc.tensor.matmul(
                        G[:, nt, :],
                        AVT[:, nt * 128 : (nt + 1) * 128],
                        WB[:],
                        start=True,
                        stop=True,
                    )
                nc.scalar.activation(
                    SIFO[:, 2:NT, :], G[:, 2:NT, 0 : 3 * H],
                    mybir.ActivationFunctionType.Sigmoid,
                )
                nc.scalar.activation(
                    TG[:, 2:NT, :], G[:, 2:NT, 3 * H : 4 * H],
                    mybir.ActivationFunctionType.Tanh,
                )
                nc.vector.scalar_tensor_tensor(
                    C[:, 0:2, :], FC[:, 0:2, :], 1.0, IG[:, 0:2, :],
                    mybir.AluOpType.bypass, mybir.AluOpType.add,
                )
                nc.gpsimd.tensor_tensor(
                    FC[:, 2:NT, :], SIFO[:, 2:NT, H : 2 * H], C[:, 2:NT, :],
                    mybir.AluOpType.mult,
                )
                nc.vector.scalar_tensor_tensor(
                    IG[:, 2:NT, :], SIFO[:, 2:NT, 0:H], 1.0, TG[:, 2:NT, :],
                    mybir.AluOpType.bypass, mybir.AluOpType.mult,
                )
                nc.scalar.activation(
                    TC[:, 0:2, :], C[:, 0:2, :], mybir.ActivationFunctionType.Tanh
                )
                nc.vector.scalar_tensor_tensor(
                    C[:, 2:NT, :], FC[:, 2:NT, :], 1.0, IG[:, 2:NT, :],
                    mybir.AluOpType.bypass, mybir.AluOpType.add,
                )
                nc.scalar.activation(
                    TC[:, 2:NT, :], C[:, 2:NT, :], mybir.ActivationFunctionType.Tanh
                )
                if s == S - 1:
                    nc.vector.scalar_tensor_tensor(
                        HF[:, 0:2, :], SIFO[:, 0:2, 2 * H : 3 * H], 1.0,
                        TC[:, 0:2, :],
                        mybir.AluOpType.bypass, mybir.AluOpType.mult,
                    )
                    nc.vector.scalar_tensor_tensor(
                        HF[:, 2:NT, :], SIFO[:, 2:NT, 2 * H : 3 * H], 1.0,
                        TC[:, 2:NT, :],
                        mybir.AluOpType.bypass, mybir.AluOpType.mult,
                    )
                else:
                    nc.vector.scalar_tensor_tensor(
                        V[:, 0:2, 0:H], SIFO[:, 0:2, 2 * H : 3 * H], 1.0,
                        TC[:, 0:2, :],
                        mybir.AluOpType.bypass, mybir.AluOpType.mult,
                    )
                    nc.vector.scalar_tensor_tensor(
                        V[:, 2:NT, 0:H], SIFO[:, 2:NT, 2 * H : 3 * H], 1.0,
                        TC[:, 2:NT, :],
                        mybir.AluOpType.bypass, mybir.AluOpType.mult,
                    )

            nc.sync.dma_start(hout_d[:], HF[:])

    nc.finalize()  # Bacc.finalize runs the multi-wait-splitting passes
    return nc


def _prep_core_inputs(b, x, adj, h0, c0, Wh, Wx, b_gates):
    bf16 = ml_dtypes.bfloat16
    # adjT[s, p, mt, n] = adj[b, s, n, mt*128+p]  (= A_s^T row m, col n)
    a = adj[b].transpose(0, 2, 1).reshape(S, NT, 128, N).transpose(0, 2, 1, 3)
    adjT = np.ascontiguousarray(a, dtype=bf16)
    # xT[p, s, mt, d] = x[b, s, mt*128+p, d]
    xb = x[b].reshape(S, NT, 128, D).transpose(2, 0, 1, 3)
    xT = np.ascontiguousarray(xb, dtype=bf16)
    # h0T/c0T[p, nt, j] = state[b, nt*128+p, j]
    h0b = h0[b].reshape(NT, 128, H).transpose(1, 0, 2)
    c0b = c0[b].reshape(NT, 128, H).transpose(1, 0, 2)
    h0T = np.ascontiguousarray(h0b, dtype=bf16)
    c0T = np.ascontiguousarray(c0b, dtype=np.float32)
    wb = np.concatenate([Wh, Wx, b_gates[None, :]], axis=0).astype(np.float32)
    wb16 = wb.astype(bf16)
    return {"adjT": adjT, "xT": xT, "wb": wb16, "h0T": h0T, "c0T": c0T}


def kernel(x, adj, initial_hidden_state, initial_cell_state, Wx, Wh, b_gates,
           W1, b1, W2, b2):
    x = np.asarray(x, dtype=np.float32)
    adj = np.asarray(adj, dtype=np.float32)
    h0 = np.asarray(initial_hidden_state, dtype=np.float32)
    c0 = np.asarray(initial_cell_state, dtype=np.float32)
    Wx_ = np.asarray(Wx, dtype=np.float32)
    Wh_ = np.asarray(Wh, dtype=np.float32)
    bg = np.asarray(b_gates, dtype=np.float32)

    if "nc" not in _CACHE:
        _CACHE["nc"] = _build_nc()
    nc = _CACHE["nc"]

    core_ids = list(range(B))
    in_maps = [_prep_core_inputs(b, x, adj, h0, c0, Wh_, Wx_, bg) for b in range(B)]
    res = run_bass_kernel_spmd(nc, in_maps, core_ids)

    h_final = np.zeros((B, N, H), dtype=np.float32)
    for i in range(B):
        hout = np.asarray(res.results[i]["hout"], dtype=np.float32)  # [128, NT, H]
        h_final[i] = hout.transpose(1, 0, 2).reshape(N, H)

    read_out = h_final[:, 0, :]  # (B, H) -- TARGET_NODE = 0
    pre = read_out @ np.asarray(W1, dtype=np.float32) + np.asarray(b1, dtype=np.float32)
    out = np.maximum(pre, 0.0) @ np.asarray(W2, dtype=np.float32) + np.asarray(
        b2, dtype=np.float32
    )
    return out.astype(np.float32)
